# revision 43
# baseline (speedup 1.0000x reference)
"""Trainium2 Bass kernel for nn_CNODExtmod (HiPPO-style continuous neural ODE).

Contract: kernel(**inputs) takes the FULL unsharded inputs (as produced by
reference.setup_inputs()) and returns the full outputs
(y_preds, y_traj, times_traj, hT_cn0), matching reference().

Strategy: pure data parallelism over the batch dim across 8 NeuronCores
(512 samples per core). Per core the state h (B, 65) is kept transposed in
SBUF with batch on the free dim, in two partition-0-based tiles:
  hc [32, B] = cn0 (HiPPO coefficients)
  hd [33, B] = cn1 (driver state)
Per Euler substep: cn0' = (I+dtA)@cn0 + dtB*cn1[0] (two accumulating
matmuls) and cn1' = cn1 + dt*MLP(cn1) (three matmuls + two relu
activations + one fused scalar_tensor_tensor). The masked observation
update builds [y_t; cn0] in PSUM via shift-matrix matmuls (engines cannot
move data across partitions; the PE can) and applies one partition-aligned
copy_predicated against a partition-broadcast mask row.
"""

import numpy as np

import concourse.bass as bass
import concourse.tile as tile
import concourse.mybir as mybir
from concourse.bass_utils import run_bass_kernel_spmd
from concourse.masks import make_identity
from concourse.tile import ScopedClock

# ---------------------------------------------------------------------------
# Problem constants (hardcoded per the task contract)
# ---------------------------------------------------------------------------
NC = 32          # HiPPO coefficients
HID = 128        # MLP hidden
DT = 0.01        # Euler substep
N_SUB = 4        # substeps per observation
T_FULL = 64      # observation steps
BATCH = 4096
NCORES = 8
BSH = BATCH // NCORES   # 512 per core
SD = 2 * NC + 1         # state dim = 65

F32 = mybir.dt.float32
F32R = mybir.dt.float32r
AF = mybir.ActivationFunctionType
ALU = mybir.AluOpType


# ---------------------------------------------------------------------------
# Workaround: this container's walrus codegen only supports a single sync
# wait on TPB_CTRL (InstDrain). Tile's tail drain carries one wait per live
# semaphore; split them into a chain of single-wait drains.
# ---------------------------------------------------------------------------
def _patched_drain_and_barrier(self, tick_clock, wait_clock):
    nc = self.nc
    drain_inst = nc.sync.drain()
    wait_clock.add_sem_waits(
        drain_inst.ins, ScopedClock({None: tick_clock.global_clock})
    )
    si = drain_inst.ins.sync_info
    waits = list(si.on_wait or []) if si else []
    if len(waits) > 1:
        drain_inst.ins.sync_info = mybir.SyncInfo(
            on_wait=[waits[0]], on_update=list(si.on_update or [])
        )
        for w in waits[1:]:
            d2 = nc.sync.drain()
            d2.ins.sync_info = mybir.SyncInfo(on_wait=[w], on_update=[])
    nc.all_engine_barrier()
    assert self.sems is not None
    popped = nc._tile_sem_poison_stack.pop()
    assert popped is self._sem_poison
    nc.clear_and_free_semaphores(list(self.sems.allocated().values()))
    nc.all_engine_barrier()


tile.TileContext._drain_and_barrier = _patched_drain_and_barrier


def _cap_sync_waits(nc, max_waits=1):
    """Walrus here encodes at most one sem wait per instruction; hoist any
    extras onto standalone same-engine EventSemaphore instructions inserted
    immediately before."""
    for fn in nc.m.functions:
        for bb in fn.blocks:
            insts = bb.instructions
            i = 0
            while i < len(insts):
                inst = insts[i]
                si = inst.sync_info
                waits = list(si.on_wait) if (si and si.on_wait) else []
                if len(waits) > max_waits:
                    extra, keep = waits[:-max_waits], waits[-max_waits:]
                    inst.sync_info = mybir.SyncInfo(
                        on_wait=keep, on_update=list(si.on_update or []))
                    for k, w in enumerate(extra):
                        ev = mybir.InstEventSemaphore(
                            name=nc.get_next_instruction_name(), ins=[], outs=[])
                        ev.engine = inst.engine
                        ev.sync_info = mybir.SyncInfo(on_wait=[w], on_update=[])
                        insts.insert(i + k, ev)
                    i += len(extra)
                i += 1


# ---------------------------------------------------------------------------
# Device program
# ---------------------------------------------------------------------------
def build_nc(T=T_FULL, use_f32r=True, n_halves=2, with_b3=True):
    """Build the per-core Bass program. Every core runs the same NEFF on its
    own 512-sample shard."""
    nc = bass.Bass("TRN2", target_bir_lowering=False, debug=False)

    MD = F32R if use_f32r else F32   # dtype of every matmul-feeding tensor

    y_d = nc.dram_tensor("y", [BSH, T], F32, kind="ExternalInput").ap()
    m_d = nc.dram_tensor("m", [BSH, T], F32, kind="ExternalInput").ap()
    # Window-folded propagator: G=I+dtA, G4=G^4, v_s=G^(3-s)dtB.
    # g4t=G4^T [32,32]; gbt4 [1,4*32] cols s*32: v_s^T; g4pt [32,33] = [0|G4^T]
    # (shift-pad: out row j gets (G4@cn0)[j-1]); pvt4 [1,4*33] shifted v_s;
    # e0 [1,33] (y into row 0); mIt [33,33] = -I.
    g4t_d = nc.dram_tensor("g4t", [NC, NC], MD, kind="ExternalInput").ap()
    gbt4_d = nc.dram_tensor("gbt4", [1, 4 * NC], MD, kind="ExternalInput").ap()
    g4pt_d = nc.dram_tensor("g4pt", [NC, NC + 1], MD, kind="ExternalInput").ap()
    pvt4_d = nc.dram_tensor("pvt4", [1, 4 * (NC + 1)], MD, kind="ExternalInput").ap()
    mIt_d = nc.dram_tensor("mIt", [NC + 1, NC + 1], MD, kind="ExternalInput").ap()
    w1t_d = nc.dram_tensor("w1t", [NC + 1, HID], MD, kind="ExternalInput").ap()
    b1_d = nc.dram_tensor("b1", [HID, 1], F32, kind="ExternalInput").ap()
    b1p_d = nc.dram_tensor("b1p", [HID, 1], F32, kind="ExternalInput").ap()
    w13t_d = nc.dram_tensor("w13t", [HID, HID], MD, kind="ExternalInput").ap()
    w1tn_d = nc.dram_tensor("w1tn", [NC + 1, HID], MD, kind="ExternalInput").ap()
    w1b3_d = nc.dram_tensor("w1b3", [1, HID], MD, kind="ExternalInput").ap()
    ones_d = nc.dram_tensor("ones", [1, BSH], MD, kind="ExternalInput").ap()
    w2t_d = nc.dram_tensor("w2t", [HID, HID], MD, kind="ExternalInput").ap()
    b2_d = nc.dram_tensor("b2", [HID, 1], F32, kind="ExternalInput").ap()
    w3t_d = nc.dram_tensor("w3t", [HID, NC + 1], MD, kind="ExternalInput").ap()
    b3_d = nc.dram_tensor("b3", [NC + 1, 1], F32, kind="ExternalInput").ap()
    e0_d = nc.dram_tensor("e0", [1, NC + 1], MD, kind="ExternalInput").ap()
    z0_d = nc.dram_tensor("z0", [NC + 1, BSH], MD, kind="ExternalInput").ap()

    preds_d = nc.dram_tensor("preds", [BSH, T], F32, kind="ExternalOutput").ap()
    hout_d = nc.dram_tensor("hout", [BSH, NC], F32, kind="ExternalOutput").ap()

    # scratch DRAM for partition-broadcast / row reload
    mT_h = nc.dram_tensor("mT_scratch", [T, BSH], F32, kind="Internal")
    mT_d = mT_h.ap()
    yT_h = nc.dram_tensor("yT_scratch", [T, BSH], MD, kind="Internal")
    yT_d = yT_h.ap()

    n_blk = BSH // 128  # 4 column blocks of 128 for transposes
    halves = [
        (i * (BSH // n_halves), (i + 1) * (BSH // n_halves))
        for i in range(n_halves)
    ]

    with tile.TileContext(nc) as tc:
        with (
            tc.tile_pool(name="consts", bufs=1) as consts,
            tc.tile_pool(name="state", bufs=1) as state,
            tc.tile_pool(name="mlp", bufs=4) as mlp,
            tc.tile_pool(name="maskb", bufs=6) as maskbp,
            tc.tile_pool(name="blendp", bufs=2) as blendp,
            tc.tile_pool(name="outp", bufs=2) as outp,
            tc.tile_pool(name="ps_g", bufs=1, space="PSUM") as ps_g,
            tc.tile_pool(name="ps_u", bufs=1, space="PSUM") as ps_u,
            tc.tile_pool(name="ps_1", bufs=1, space="PSUM") as ps_1,
            tc.tile_pool(name="ps_2", bufs=1, space="PSUM") as ps_2,
        ):
            ident = consts.tile([128, 128], F32)
            make_identity(nc, ident)

            g4t_sb = consts.tile([NC, NC], MD)
            gbt4_sb = consts.tile([1, 4 * NC], MD)
            g4pt_sb = consts.tile([NC, NC + 1], MD)
            pvt4_sb = consts.tile([1, 4 * (NC + 1)], MD)
            mIt_sb = consts.tile([NC + 1, NC + 1], MD)
            w1t_sb = consts.tile([NC + 1, HID], MD)
            b1_sb = consts.tile([HID, 1], F32)
            b1p_sb = consts.tile([HID, 1], F32)
            w13t_sb = consts.tile([HID, HID], MD)
            w1tn_sb = consts.tile([NC + 1, HID], MD)
            w1b3_sb = consts.tile([1, HID], MD)
            ones_sb = consts.tile([1, BSH], MD)
            w2t_sb = consts.tile([HID, HID], MD)
            b2_sb = consts.tile([HID, 1], F32)
            w3t_sb = consts.tile([HID, NC + 1], MD)
            b3_sb = consts.tile([NC + 1, 1], F32)
            e0_sb = consts.tile([1, NC + 1], MD)
            for sb, d in (
                (g4t_sb, g4t_d), (gbt4_sb, gbt4_d), (g4pt_sb, g4pt_d),
                (pvt4_sb, pvt4_d), (mIt_sb, mIt_d), (w1t_sb, w1t_d),
                (b1_sb, b1_d), (b1p_sb, b1p_d), (w13t_sb, w13t_d),
                (w1tn_sb, w1tn_d), (w1b3_sb, w1b3_d), (ones_sb, ones_d),
                (w2t_sb, w2t_d), (b2_sb, b2_d),
                (w3t_sb, w3t_d), (b3_sb, b3_d), (e0_sb, e0_d),
            ):
                nc.sync.dma_start(sb[:], d[:])

            # ---- load Y and mask, transpose to [T, BSH] via PE ----
            yT = consts.tile([T, BSH], MD)
            mTs = consts.tile([T, BSH], F32)
            for src_d, dst in ((y_d, yT), (m_d, mTs)):
                for b in range(n_blk):
                    blk = outp.tile([128, T], F32, tag="ld")
                    nc.sync.dma_start(blk[:], src_d[b * 128:(b + 1) * 128, :])
                    pt = ps_1.tile([T, 128], F32, tag="p1_0")
                    nc.tensor.transpose(pt[:], blk[:], ident[:])
                    nc.scalar.copy(dst[0:T, b * 128:(b + 1) * 128], pt[:])
            # transposed copies to DRAM for per-row / broadcast reload
            nc.sync.dma_start(mT_d[:], mTs[:])
            nc.sync.dma_start(yT_d[:], yT[:])

            preds_sb = consts.tile([T, BSH], F32)

            # ---- per-half persistent state (ping-pong) ----
            hst = []
            flats = []
            for hf, (c0, c1) in enumerate(halves):
                w = c1 - c0
                hc = state.tile([NC, w], MD, name=f"hc_{hf}", tag=f"hc_{hf}")
                hd = [state.tile([NC + 1, w], MD, name=f"hd{i}_{hf}",
                                 tag=f"hd{i}_{hf}") for i in range(2)]
                nc.sync.dma_start(hc[:], z0_d[0:NC, c0:c1])
                nc.sync.dma_start(hd[0][:], z0_d[:, c0:c1])
                hst.append((hc, hd))
                flats.append(state.tile([1, T * w], F32, name=f"pf_{hf}",
                                        tag=f"pf_{hf}"))

            def substep_tiles(hf, s):
                hc, hd = hst[hf]
                c0, c1 = halves[hf]
                w = c1 - c0
                return dict(
                    cur_d=hd[s % 2], nxt_d=hd[(s + 1) % 2],
                    p1=ps_1.tile([HID, w], F32, tag="p1", name=f"p1_{hf}_{s}"),
                    p2=ps_2.tile([HID, w], F32, tag="p2", name=f"p2_{hf}_{s}"),
                    po3=ps_2.tile([NC + 1, w], F32, tag="p2",
                                  name=f"po3_{hf}_{s}"),
                    t1=mlp.tile([HID, w], MD, tag=f"t1_{hf}",
                                name=f"t1_{hf}_{s}"),
                    t2=mlp.tile([HID, w], MD, tag=f"t2_{hf}",
                                name=f"t2_{hf}_{s}"),
                )

            # Anti-phased software pipeline: the per-substep stage engine
            # sequence is [PE, ACT, PE, ACT, PE, DVE]; running the second
            # batch-half 3 stages behind the first gives collision-free
            # engine usage in steady state (PE<->ACT/DVE always opposite).
            shared = {}

            def get_shared(t):
                if t not in shared:
                    mb = maskbp.tile([NC + 1, BSH], F32, tag="mb",
                                     name=f"mb_{t}")
                    nc.sync.dma_start(
                        mb[:], bass.AP(tensor=mT_h, offset=t * BSH,
                                       ap=[[0, NC + 1], [1, BSH]]))
                    yb = maskbp.tile([1, BSH], MD, tag="yb", name=f"yb_{t}")
                    nc.sync.dma_start(yb[:], yT_d[t:t + 1, :])
                    shared[t] = (mb, yb)
                return shared[t]

            def half_program(hf):
                c0, c1 = halves[hf]
                w = c1 - c0
                hc, hd = hst[hf]
                pend = [None]
                for t in range(T):
                    win_state = {}

                    def win_start(t=t, win_state=win_state):
                        get_shared(t)
                        poG = ps_g.tile([NC, w], F32, tag=f"poG_{hf}",
                                        name=f"poG_{hf}")
                        pu = ps_u.tile([NC + 1, w], F32, tag=f"pu_{hf}",
                                       name=f"pu_{hf}")
                        if pend[0] is None:
                            B1 = ps_1.tile([HID, w], F32, tag=f"p1_{hf}",
                                           name=f"p1_{hf}")
                            nc.tensor.matmul(B1[:], w1t_sb[:], hd[0][:],
                                             start=True, stop=False)
                        else:
                            B1 = pend[0]
                            pend[0] = None
                        win_state["pw"] = (poG, pu)
                        win_state["B1"] = B1

                    yield win_start
                    for st in range(N_SUB):
                        tl = {}

                        def s1_act1(t=t, st=st, tl=tl, win_state=win_state):
                            tl["t1"] = mlp.tile([HID, w], MD, tag=f"t1_{hf}",
                                                name=f"t1_{hf}")
                            nc.scalar.activation(
                                tl["t1"][:], win_state["B1"][:], AF.Relu,
                                bias=b1_sb[:], scale=1.0)

                        def s2_mm2(t=t, st=st, tl=tl, win_state=win_state):
                            poG, pu = win_state["pw"]
                            cur_d = hd[st % 2]
                            tl["p2"] = ps_2.tile([HID, w], F32,
                                                 tag=f"p2_{hf}",
                                                 name=f"p2_{hf}")
                            nc.tensor.matmul(tl["p2"][:], w2t_sb[:],
                                             tl["t1"][:],
                                             start=True, stop=True)
                            if st == 0:
                                _, yb = get_shared(t)
                                nc.tensor.matmul(poG[:], g4t_sb[:], hc[:],
                                                 start=True, stop=False)
                                nc.tensor.matmul(pu[:], g4pt_sb[:], hc[:],
                                                 start=True, stop=False)
                                nc.tensor.matmul(pu[:], e0_sb[:],
                                                 yb[0:1, c0:c1],
                                                 start=False, stop=False)
                            nc.tensor.matmul(
                                poG[:], gbt4_sb[0:1, st * NC:(st + 1) * NC],
                                cur_d[0:1, :], start=False,
                                stop=(st == N_SUB - 1))
                            nc.tensor.matmul(
                                pu[:],
                                pvt4_sb[0:1,
                                        st * (NC + 1):(st + 1) * (NC + 1)],
                                cur_d[0:1, :], start=False,
                                stop=(st == N_SUB - 1))

                        def s3_act2(t=t, st=st, tl=tl, win_state=win_state):
                            tl["t2"] = mlp.tile([HID, w], MD, tag=f"t2_{hf}",
                                                name=f"t2_{hf}")
                            nc.scalar.activation(tl["t2"][:], tl["p2"][:],
                                                 AF.Relu, bias=b2_sb[:],
                                                 scale=1.0)
                            if st == N_SUB - 1 and t < T - 1:
                                mb, _ = get_shared(t)
                                _, pu = win_state["pw"]
                                ma = blendp.tile([NC + 1, w], MD,
                                                 tag=f"ma_{hf}",
                                                 name=f"ma_{hf}")
                                nc.vector.tensor_mul(ma[:], pu[:],
                                                     mb[0:NC + 1, c0:c1])
                                win_state["ma"] = ma

                        def s4_next(t=t, st=st, tl=tl, win_state=win_state):
                            last = st == N_SUB - 1
                            if not last:
                                nc.tensor.matmul(
                                    win_state["B1"][:], w13t_sb[:],
                                    tl["t2"][:], start=False,
                                    stop=(st == N_SUB - 2))
                                if with_b3:
                                    nc.tensor.matmul(
                                        win_state["B1"][:], w1b3_sb[:],
                                        ones_sb[0:1, c0:c1],
                                        start=False, stop=False)
                            tl["po3"] = ps_2.tile([NC + 1, w], F32,
                                                  tag=f"p2_{hf}",
                                                  name=f"po3_{hf}")
                            nc.tensor.matmul(tl["po3"][:], w3t_sb[:],
                                             tl["t2"][:],
                                             start=True, stop=True)
                            nc.vector.scalar_tensor_tensor(
                                out=hd[(st + 1) % 2][:], in0=tl["po3"][:],
                                scalar=b3_sb[:], in1=hd[st % 2][:],
                                op0=ALU.add, op1=ALU.add)


                        yield s1_act1
                        yield s2_mm2
                        yield s3_act2
                        yield s4_next

                    if t < T - 1:
                        def win_blend(t=t, win_state=win_state):
                            # m * cn1_post (all-SBUF: DVE 2x mode) — the only
                            # blend piece behind the last stt
                            mb, _ = get_shared(t)
                            mc = blendp.tile([NC + 1, w], MD,
                                             tag=f"mc_{hf}", name=f"mc_{hf}")
                            nc.vector.tensor_mul(mc[:], hd[0][:],
                                                 mb[0:NC + 1, c0:c1])
                            win_state["mc"] = mc

                        def win_p1n(t=t, win_state=win_state):
                            ma, mc = win_state["ma"], win_state["mc"]
                            p1n = ps_1.tile([HID, w], F32, tag=f"p1_{hf}",
                                            name=f"p1n_{hf}")
                            nc.tensor.matmul(p1n[:], w1t_sb[:], hd[0][:],
                                             start=True, stop=False)
                            nc.tensor.matmul(p1n[:], w1t_sb[:], ma[:],
                                             start=False, stop=False)
                            nc.tensor.matmul(p1n[:], w1tn_sb[:], mc[:],
                                             start=False, stop=True)
                            pend[0] = p1n

                        def win_end_copy(t=t, win_state=win_state):
                            ma, mc = win_state["ma"], win_state["mc"]
                            nc.gpsimd.tensor_copy(
                                flats[hf][0:1, t * w:(t + 1) * w],
                                hd[0][0:1, :].bitcast(F32))
                            nc.vector.tensor_tensor(
                                out=hd[0][:], in0=hd[0][:], in1=ma[:],
                                op=ALU.add)
                            nc.vector.tensor_sub(hd[0][:], hd[0][:], mc[:])
                            nc.vector.tensor_copy(hc[:], win_state["pw"][0][:])

                        yield win_blend
                        yield win_p1n
                        yield win_end_copy
                    else:
                        def win_end_copy(t=t, win_state=win_state):
                            nc.gpsimd.tensor_copy(
                                flats[hf][0:1, t * w:(t + 1) * w],
                                hd[0][0:1, :].bitcast(F32))
                            nc.vector.tensor_copy(hc[:], win_state["pw"][0][:])

                        yield win_end_copy

            SKEW = 3
            progs = [list(half_program(hf)) for hf in range(n_halves)]
            if n_halves == 2:
                a, b = progs
                order = []
                for i in range(len(a) + SKEW):
                    if i < len(a):
                        order.append(a[i])
                    if i - SKEW >= 0 and i - SKEW < len(b):
                        order.append(b[i - SKEW])
                for fn in order:
                    fn()
            else:
                for prog in progs:
                    for fn in prog:
                        fn()

            # ---- preds: flats -> [T, BSH] -> transpose -> (BSH, T) ----
            for hf, (c0, c1) in enumerate(halves):
                w = c1 - c0
                nc.sync.dma_start(preds_sb[0:T, c0:c1], flats[hf][0:1, :])
            for b in range(n_blk):
                pt = ps_1.tile([128, T], F32, tag="p1_0")
                nc.tensor.transpose(pt[:], preds_sb[0:T, b * 128:(b + 1) * 128],
                                    ident[0:T, 0:T])
                ob = outp.tile([128, T], F32, tag="ob")
                nc.scalar.copy(ob[:], pt[:])
                nc.sync.dma_start(preds_d[b * 128:(b + 1) * 128, :], ob[:])

            # ---- hout: cn0 of final state, transposed ----
            per_half_blk = max(1, n_blk // n_halves)
            for b in range(n_blk):
                hf = min(b // per_half_blk, n_halves - 1)
                lb = b - hf * per_half_blk
                hc0 = hst[hf][0]
                pt = ps_1.tile([128, NC], F32, tag="p1_0")
                nc.tensor.transpose(
                    pt[:], hc0[0:NC, lb * 128:(lb + 1) * 128].bitcast(F32),
                    ident[0:NC, 0:NC])
                ob = outp.tile([128, NC], F32, tag="oh")
                nc.scalar.copy(ob[:], pt[:])
                nc.sync.dma_start(hout_d[b * 128:(b + 1) * 128, :], ob[:])

    _cap_sync_waits(nc)
    return nc


# ---------------------------------------------------------------------------
# Host side
# ---------------------------------------------------------------------------
_CACHE = {}


def _get_nc(T=T_FULL, use_f32r=True, n_halves=2, with_b3=True):
    key = (T, use_f32r, n_halves, with_b3)
    if key not in _CACHE:
        _CACHE[key] = build_nc(T=T, use_f32r=use_f32r, n_halves=n_halves,
                               with_b3=with_b3)
    return _CACHE[key]


def run_device(times, Y, mask, A, Bvec, W1, b1, W2, b2, W3, b3,
               T=T_FULL, use_f32r=True, n_halves=2, **rkw):
    f = np.float32
    A64 = np.asarray(A, np.float64)
    G = np.eye(NC) + DT * A64
    G4 = np.linalg.matrix_power(G, 4)
    dtB = DT * np.asarray(Bvec, np.float64)
    vs = [np.linalg.matrix_power(G, 3 - s) @ dtB for s in range(4)]
    gbt4 = np.concatenate([v[None, :] for v in vs], axis=1)        # [1, 4*32]
    g4pt = np.concatenate([np.zeros((NC, 1)), G4.T], axis=1)       # [32, 33]
    pvt4 = np.concatenate(
        [np.concatenate([[0.0], v])[None, :] for v in vs], axis=1)  # [1, 4*33]
    wk = dict(
        g4t=np.ascontiguousarray(G4.T, f),
        gbt4=np.ascontiguousarray(gbt4, f),
        g4pt=np.ascontiguousarray(g4pt, f),
        pvt4=np.ascontiguousarray(pvt4, f),
        mIt=np.ascontiguousarray(-np.eye(NC + 1, dtype=f)),
        w1t=np.ascontiguousarray(np.asarray(W1, f).T),
        b1=np.ascontiguousarray(np.asarray(b1, f)[:, None]),
        b1p=np.ascontiguousarray(
            (np.asarray(b1, np.float64)
             + np.asarray(W1, np.float64) @ (DT * np.asarray(b3, np.float64))
             )[:, None].astype(f)),
        w1tn=np.ascontiguousarray(-np.asarray(W1, f).T),
        w1b3=np.ascontiguousarray(
            (np.asarray(W1, np.float64)
             @ (DT * np.asarray(b3, np.float64)))[None, :].astype(f)),
        ones=np.ones((1, BSH), f),
        w13t=np.ascontiguousarray(
            (np.asarray(W1, np.float64)
             @ (DT * np.asarray(W3, np.float64))).T.astype(f)),
        w2t=np.ascontiguousarray(np.asarray(W2, f).T),
        b2=np.ascontiguousarray(np.asarray(b2, f)[:, None]),
        w3t=np.ascontiguousarray((f(DT) * np.asarray(W3, f)).T),
        b3=np.ascontiguousarray((f(DT) * np.asarray(b3, f))[:, None]),
        e0=np.ascontiguousarray(np.eye(1, NC + 1, dtype=f)),
        z0=np.zeros((NC + 1, BSH), f),
    )
    Yf = np.asarray(Y, f)[:, :T, 0]
    Mf = np.asarray(mask, f)[:, :T]
    in_maps = []
    for c in range(NCORES):
        sl = slice(c * BSH, (c + 1) * BSH)
        in_maps.append(dict(y=np.ascontiguousarray(Yf[sl]),
                            m=np.ascontiguousarray(Mf[sl]), **wk))
    nc = _get_nc(T=T, use_f32r=use_f32r, n_halves=n_halves,
                 with_b3=bool(np.any(np.asarray(b3) != 0)))
    last_exc = None
    for _attempt in range(3):
        try:
            res = run_bass_kernel_spmd(nc, in_maps,
                                       core_ids=list(range(NCORES)), **rkw)
            break
        except Exception as e:   # transient NRT_EXEC_UNIT_UNRECOVERABLE etc.
            last_exc = e
    else:
        raise last_exc
    preds = np.concatenate([res.results[c]["preds"] for c in range(NCORES)], 0)
    hout = np.concatenate([res.results[c]["hout"] for c in range(NCORES)], 0)
    return preds, hout, res


def kernel(times, Y, mask, A, Bvec, W1, b1, W2, b2, W3, b3):
    preds, hout, _ = run_device(times, Y, mask, A, Bvec, W1, b1, W2, b2, W3, b3)
    y_preds = np.ascontiguousarray(preds[:, :, None], np.float32)
    times_out = np.asarray(times, np.float32)
    return y_preds, y_preds.copy(), times_out, np.ascontiguousarray(hout, np.float32)
